# revision 1
# baseline (speedup 1.0000x reference)
"""ColPali MQA attention block on 8 Trainium2 NeuronCores.

The reference contains the ColPali reshape quirk: the attention output
[B, H, L, 1, D] is reshaped row-major straight to [B, L, H*D], which mixes
heads and positions.  Output row l' therefore depends ONLY on head
h = l'//256, gathering positions (l'%256)*8 + j for j in 0..7:

    Y[b, l', e] = sum_{j,d} O[b, l'//256, (l'%256)*8+j, d] * Wo[e, j*256+d]

Sharding: core c -> batch b=c//4 and heads {h0, h0+1} with h0=2*(c%4).
Each core computes K/V projection for its batch (replicated inside the
4-core batch group), Q projection + attention for its 2 heads over the full
sequence, and o_proj for output rows [256*h0, 256*h0+512).  Per-core outputs
are disjoint [512, 2048] slices of the [2, 2048, 2048] output -> no
cross-core communication.

Layouts (contraction dim always on SBUF partitions; zero on-device
transposes):
  - q, k produced transposed ([D, L]) by making W the stationary operand.
  - v produced natural ([L, D]) by making X the stationary operand.
  - scores computed transposed: S^T[lk, lq] = k @ q_h^T, so the exp output
    P^T[lk, lq] directly feeds O^T[d, lq] = v^T @ P^T as moving operand.
  - softmax row sums over lk (= partitions) via an all-ones [128,128]
    stationary matmul, which lands the sums pre-broadcast across all 128
    partitions; one reciprocal gives the scale tile directly.  The
    normalize multiply writes through a (u w)->(w u) access pattern that
    performs the ColPali gather for free, producing G[c, r] = O^T[d, r*8+j]
    (c = j*256+d) which is directly the stationary operand of o_proj.
Matmul inputs are bf16 (PE runs 2x faster than f32); accumulation is f32.
X^T is DMA'd in 512-column blocks with the projection psum groups
accumulated e-outer, so the PE gets matmuls per arriving chunk and ramps
immediately instead of waiting for the full X^T load; Wk streams ahead of
Wq/Wv so the k-projection starts first.
"""

import numpy as np

import concourse.mybir as mybir
import concourse.tile as tile
from concourse import bacc
from concourse.bass_utils import run_bass_kernel_spmd

F32 = mybir.dt.float32
BF16 = mybir.dt.bfloat16
AF = mybir.ActivationFunctionType
OP = mybir.AluOpType

B, L, H, D, E = 2, 2048, 8, 256, 2048
HD = H * D  # 2048
P = 128
EC = E // P  # 16 e-chunks
LT = L // P  # 16 l-tiles
SCALING = D ** -0.5  # 1/16
N_CORES = 8


def build_program():
    nc = bacc.Bacc("TRN2", target_bir_lowering=False, debug=False,
                   num_devices=N_CORES)

    xt = nc.dram_tensor("xt", [E, L], F32, kind="ExternalInput").ap()
    cost = nc.dram_tensor("cost", [D, L], F32, kind="ExternalInput").ap()
    sint = nc.dram_tensor("sint", [D, L], F32, kind="ExternalInput").ap()
    wqt = nc.dram_tensor("wqt", [E, 2 * D], F32, kind="ExternalInput").ap()
    wkt = nc.dram_tensor("wkt", [E, D], F32, kind="ExternalInput").ap()
    wvt = nc.dram_tensor("wvt", [E, D], F32, kind="ExternalInput").ap()
    wot = nc.dram_tensor("wot", [HD, E], F32, kind="ExternalInput").ap()
    out = nc.dram_tensor("out", [4 * P, E], F32, kind="ExternalOutput").ap()

    with tile.TileContext(nc) as tc:
        with tc.tile_pool(name="res", bufs=1) as res:
            kT = [res.tile([P, L], BF16, tag=f"kT{i}", name=f"kT{i}")
                  for i in range(2)]
            v_bf = [res.tile([P, D], BF16, tag=f"v{i}", name=f"v{i}")
                    for i in range(LT)]
            # q^T for the core's two heads: 4 dq-tiles x [128, L]
            qT = [res.tile([P, L], BF16, tag=f"qT{i}", name=f"qT{i}")
                  for i in range(4)]
            ones128 = res.tile([P, P], BF16, tag="ones128", name="ones128")
            nc.vector.memset(ones128[:], 1.0)

            # ---------------- Phase 1: projections + RoPE ----------------
            with tc.tile_pool(name="proj", bufs=1) as proj, \
                 tc.tile_pool(name="proj_ps", space="PSUM", bufs=1) as pps:
                xt_bf = [proj.tile([P, L], BF16, tag=f"xt{e}", name=f"xtbf{e}")
                         for e in range(EC)]
                wkt_bf = [proj.tile([P, D], BF16, tag=f"wkt{e}",
                                    name=f"wktbf{e}") for e in range(EC)]
                wvt_bf = [proj.tile([P, D], BF16, tag=f"wvt{e}",
                                    name=f"wvtbf{e}") for e in range(EC)]
                wqt_bf = [proj.tile([P, 2 * D], BF16, tag=f"wqt{e}",
                                    name=f"wqtbf{e}") for e in range(EC)]

                def load_xt_col(e, lc):
                    sl = slice(lc * 512, (lc + 1) * 512)
                    xcst = proj.tile([P, 512], F32, tag="xcst", bufs=10,
                                     name=f"xcst{e}_{lc}")
                    nc.sync.dma_start(out=xcst[:],
                                      in_=xt[e * P:(e + 1) * P, sl])
                    nc.vector.tensor_copy(xt_bf[e][:, sl], xcst[:])

                def load_csk(lc, store):
                    sl = slice(lc * 512, (lc + 1) * 512)
                    for nm, srcd in (("cos", cost), ("sin", sint)):
                        for half in range(2):
                            t = proj.tile([P, 512], F32, tag="csk", bufs=8,
                                          name=f"k{nm}{half}_{lc}")
                            nc.sync.dma_start(
                                out=t[:],
                                in_=srcd[half * P:(half + 1) * P, sl])
                            store[(nm, half)] = t

                # Wk + first X^T column-block, interleaved per e-chunk so
                # the first k-projection matmuls are enabled immediately.
                for e in range(EC):
                    wkst = proj.tile([P, D], F32, tag="wkst", bufs=5,
                                     name=f"wkst{e}")
                    nc.sync.dma_start(out=wkst[:],
                                      in_=wkt[e * P:(e + 1) * P, :])
                    nc.vector.tensor_copy(wkt_bf[e][:], wkst[:])
                    load_xt_col(e, 0)
                csks = [dict() for _ in range(4)]
                load_csk(0, csks[0])
                # Wq/Wv stream next (needed ~15us in).
                for e in range(EC):
                    wqvst = proj.tile([P, 3 * D], F32, tag="wqvst", bufs=3,
                                      name=f"wqvst{e}")
                    nc.sync.dma_start(out=wqvst[:, :2 * D],
                                      in_=wqt[e * P:(e + 1) * P, :])
                    nc.sync.dma_start(out=wqvst[:, 2 * D:],
                                      in_=wvt[e * P:(e + 1) * P, :])
                    nc.scalar.copy(wqt_bf[e][:], wqvst[:, :2 * D])
                    nc.scalar.copy(wvt_bf[e][:], wqvst[:, 2 * D:])

                # K and Q projections + RoPE + V projection, per l-chunk.
                for lc in range(4):
                    sl = slice(lc * 512, (lc + 1) * 512)
                    if lc > 0:
                        for e in range(EC):
                            load_xt_col(e, lc)
                        load_csk(lc, csks[lc])
                    csk = csks[lc]

                    pk0 = pps.tile([P, 512], F32, tag="pk", bufs=2,
                                   name=f"pk0_{lc}")
                    pk1 = pps.tile([P, 512], F32, tag="pk", bufs=2,
                                   name=f"pk1_{lc}")
                    for e in range(EC):
                        st, sp = (e == 0), (e == EC - 1)
                        xs = xt_bf[e][:, sl]
                        nc.tensor.matmul(pk0[:], wkt_bf[e][:, 0:P], xs,
                                         start=st, stop=sp)
                        nc.tensor.matmul(pk1[:], wkt_bf[e][:, P:2 * P], xs,
                                         start=st, stop=sp)

                    def _rope(p0, p1, out0, out1, tag):
                        ta = proj.tile([P, 512], F32, tag="ropetmp", bufs=4,
                                       name=f"ta{tag}")
                        tb = proj.tile([P, 512], F32, tag="ropetmp", bufs=4,
                                       name=f"tb{tag}")
                        nc.vector.tensor_tensor(ta[:], p0[:],
                                                csk[("cos", 0)][:], OP.mult)
                        nc.vector.tensor_tensor(tb[:], p1[:],
                                                csk[("sin", 0)][:], OP.mult)
                        nc.vector.tensor_tensor(out0, ta[:], tb[:],
                                                OP.subtract)
                        tc2 = proj.tile([P, 512], F32, tag="ropetmp", bufs=4,
                                        name=f"tc{tag}")
                        td = proj.tile([P, 512], F32, tag="ropetmp", bufs=4,
                                       name=f"td{tag}")
                        nc.vector.tensor_tensor(tc2[:], p1[:],
                                                csk[("cos", 1)][:], OP.mult)
                        nc.vector.tensor_tensor(td[:], p0[:],
                                                csk[("sin", 1)][:], OP.mult)
                        nc.vector.tensor_tensor(out1, tc2[:], td[:], OP.add)

                    _rope(pk0, pk1, kT[0][:, sl], kT[1][:, sl], f"k{lc}")

                    pq = [pps.tile([P, 512], F32, tag=f"pq{j}", bufs=1,
                                   name=f"pq{lc}_{j}") for j in range(4)]
                    for e in range(EC):
                        st, sp = (e == 0), (e == EC - 1)
                        xs = xt_bf[e][:, sl]
                        for j in range(4):
                            nc.tensor.matmul(pq[j][:],
                                             wqt_bf[e][:, j * P:(j + 1) * P],
                                             xs, start=st, stop=sp)
                    _rope(pq[0], pq[1], qT[0][:, sl], qT[1][:, sl], f"q0{lc}")
                    _rope(pq[2], pq[3], qT[2][:, sl], qT[3][:, sl], f"q1{lc}")

                    # V projection for this chunk's four l-tiles.
                    for lt in range(4 * lc, 4 * lc + 4):
                        pv = pps.tile([P, D], F32, tag="pv", bufs=2,
                                      name=f"pv{lt}")
                        for e in range(EC):
                            nc.tensor.matmul(pv[:],
                                             xt_bf[e][:, lt * P:(lt + 1) * P],
                                             wvt_bf[e][:],
                                             start=(e == 0),
                                             stop=(e == EC - 1))
                        nc.vector.tensor_copy(v_bf[lt][:], pv[:])

            # ------------- Phase 2: attention + o_proj -------------
            with tc.tile_pool(name="att", bufs=1) as att, \
                 tc.tile_pool(name="att_ps", space="PSUM", bufs=1) as aps:
                # G[hh][half]: gathered, normalized O^T.  G[c-row, col] with
                # c-row = d within half, column layout j*256 + r.
                G = [[att.tile([P, L], BF16, tag=f"G{hh}{dt}",
                               name=f"G{hh}{dt}") for dt in range(2)]
                     for hh in range(2)]
                wot_bf = [att.tile([P, E], BF16, tag=f"wot{i}",
                                   name=f"wotbf{i}") for i in range(EC)]
                for i in range(EC):
                    wost = att.tile([P, E], F32, tag="wostage", bufs=2,
                                    name=f"wost{i}")
                    nc.sync.dma_start(out=wost[:],
                                      in_=wot[i * P:(i + 1) * P, :])
                    eng = nc.vector if i % 2 == 0 else nc.scalar
                    (eng.tensor_copy if i % 2 == 0 else eng.copy)(
                        wot_bf[i][:], wost[:])

                def o_proj(a_idx):
                    for rh in range(2):
                        rt = a_idx * 2 + rh
                        for eg in range(4):
                            esl = slice(eg * 512, (eg + 1) * 512)
                            py = aps.tile([P, 512], F32, tag="py", bufs=2,
                                          name=f"py{rt}_{eg}")
                            for m in range(EC):
                                lhsT = G[a_idx][m % 2][
                                    :, (m // 2) * 256 + rh * P:
                                       (m // 2) * 256 + rh * P + P]
                                nc.tensor.matmul(py[:], lhsT,
                                                 wot_bf[m][:, esl],
                                                 start=(m == 0),
                                                 stop=(m == EC - 1))
                            ysb = att.tile([P, 512], F32, tag="ysb", bufs=3,
                                           name=f"ysb{rt}_{eg}")
                            nc.scalar.copy(ysb[:], py[:])
                            nc.sync.dma_start(
                                out=out[rt * P:(rt + 1) * P, esl],
                                in_=ysb[:])

                for hh in range(2):
                    qh0, qh1 = qT[2 * hh], qT[2 * hh + 1]
                    for lqc in range(4):
                        qsl = slice(lqc * 512, (lqc + 1) * 512)
                        pt = [att.tile([P, 512], BF16, tag=f"pt{i}", bufs=2,
                                       name=f"pt{hh}_{lqc}_{i}")
                              for i in range(LT)]
                        for lk in range(LT):
                            ps = aps.tile([P, 512], F32, tag="ps", bufs=3,
                                          name=f"ps{hh}_{lqc}_{lk}")
                            nc.tensor.matmul(ps[:],
                                             kT[0][:, lk * P:(lk + 1) * P],
                                             qh0[:, qsl],
                                             start=True, stop=False)
                            nc.tensor.matmul(ps[:],
                                             kT[1][:, lk * P:(lk + 1) * P],
                                             qh1[:, qsl],
                                             start=False, stop=True)
                            nc.scalar.activation(pt[lk][:], ps[:], AF.Exp,
                                                 scale=float(SCALING))
                        # Row sums, pre-broadcast over all 128 partitions
                        # by the all-ones stationary operand.
                        prb = aps.tile([P, 512], F32, tag="prb", bufs=1,
                                       name=f"prb{hh}_{lqc}")
                        for lk in range(LT):
                            nc.tensor.matmul(prb[:], ones128[:], pt[lk][:],
                                             start=(lk == 0),
                                             stop=(lk == LT - 1))
                        rb = att.tile([P, 512], F32, tag="rb", bufs=2,
                                      name=f"rb{hh}_{lqc}")
                        nc.vector.reciprocal(rb[:], prb[:])
                        rb_wu = rb.rearrange("p (u w) -> p w u", w=8)
                        for dt in range(2):
                            po = aps.tile([P, 512], F32, tag="po", bufs=2,
                                          name=f"po{hh}_{lqc}_{dt}")
                            for lk in range(LT):
                                nc.tensor.matmul(
                                    po[:],
                                    v_bf[lk][:, dt * P:(dt + 1) * P],
                                    pt[lk][:],
                                    start=(lk == 0), stop=(lk == LT - 1))
                            # normalize + ColPali gather in one op:
                            # G[:, j*256 + 64*lqc + u] = po[:, 8u+j]*rb[:, 8u+j]
                            g_dst = G[hh][dt].rearrange(
                                "p (w r) -> p w r",
                                w=8)[:, :, 64 * lqc:64 * lqc + 64]
                            nc.vector.tensor_tensor(
                                g_dst,
                                po.rearrange("p (u w) -> p w u", w=8),
                                rb_wu, OP.mult)
                    o_proj(hh)

    nc.compile()
    return nc


_NC = None


def _get_nc():
    global _NC
    if _NC is None:
        _NC = build_program()
    return _NC


def make_in_maps(hidden_states, cos, sin, Wq, Wk, Wv, Wo):
    hs = np.asarray(hidden_states, np.float32)
    xt = [np.ascontiguousarray(hs[b].T) for b in range(B)]
    cost = np.ascontiguousarray(np.asarray(cos, np.float32).T)
    sint = np.ascontiguousarray(np.asarray(sin, np.float32).T)
    wqt = np.ascontiguousarray(np.asarray(Wq, np.float32).T)
    wkt = np.ascontiguousarray(np.asarray(Wk, np.float32).T)
    wvt = np.ascontiguousarray(np.asarray(Wv, np.float32).T)
    wot = np.ascontiguousarray(np.asarray(Wo, np.float32).T)
    in_maps = []
    for c in range(N_CORES):
        b, ql = c // 4, c % 4
        in_maps.append({
            "xt": xt[b],
            "cost": cost,
            "sint": sint,
            "wqt": np.ascontiguousarray(wqt[:, ql * 512:(ql + 1) * 512]),
            "wkt": wkt,
            "wvt": wvt,
            "wot": wot,
        })
    return in_maps


def assemble(results):
    y = np.empty((B, L, E), np.float32)
    for c in range(N_CORES):
        b, ql = c // 4, c % 4
        y[b, ql * 512:(ql + 1) * 512, :] = results[c]["out"]
    return y


def kernel(hidden_states, attention_mask, cos, sin, Wq, Wk, Wv, Wo):
    # attention_mask is additive and all-zero per the problem spec; it is
    # accepted for signature compatibility but not shipped to the device.
    nc = _get_nc()
    in_maps = make_in_maps(hidden_states, cos, sin, Wq, Wk, Wv, Wo)
    res = run_bass_kernel_spmd(nc, in_maps, core_ids=list(range(N_CORES)))
    return assemble(res.results)



# revision 5
# speedup vs baseline: 1.0346x; 1.0346x over previous
"""ColPali MQA attention block on 8 Trainium2 NeuronCores.

The reference contains the ColPali reshape quirk: the attention output
[B, H, L, 1, D] is reshaped row-major straight to [B, L, H*D], which mixes
heads and positions.  Output row l' therefore depends ONLY on head
h = l'//256, gathering positions (l'%256)*8 + j for j in 0..7.

Sharding: core c -> batch b=c//4 and heads {h0, h0+1} with h0=2*(c%4).
K/V projection is deduplicated inside each 4-core batch group: core c
computes K^T/V only for its own 512 sequence positions (ql = c%4), then a
4-rank AllGather (on TOPSP/SDMA hardware, overlapped with the Q projection
on the PE) assembles the full K^T and V on every core.

All inputs are pre-converted to bf16 on the host and DMA'd straight into
their SBUF layouts (no on-device staging casts).  The softmax row-sum is
computed as a pairwise bf16 add-tree on the vector engine plus a single
all-ones matmul for the cross-partition reduction (instead of 16 PE
matmuls per attention block); 1/rowsum uses the fast approximate DVE
reciprocal (~18 bits, plenty against the 2e-2 gate).

Phase 2 is software-pipelined: the scores+exp stream for attention block N
is interleaved (2 matmuls per score-pair) with a FIFO backlog of deferred
PE work (PV accumulation of block N-1, rowsum reduce, o_proj chains), so
the PE never idles waiting for the scalar engine's exp drain.  o_proj
results are DMA'd directly from PSUM to the output DRAM tensor.
"""

import contextlib

import numpy as np
import ml_dtypes

import concourse.mybir as mybir
import concourse.tile as tile
from concourse import bacc
from concourse.bass_utils import run_bass_kernel_spmd

F32 = mybir.dt.float32
BF16 = mybir.dt.bfloat16
AF = mybir.ActivationFunctionType
OP = mybir.AluOpType

B, L, H, D, E = 2, 2048, 8, 256, 2048
HD = H * D  # 2048
P = 128
EC = E // P  # 16 e-chunks
LT = L // P  # 16 l-tiles
SCALING = D ** -0.5  # 1/16
N_CORES = 8

USE_COLLECTIVE = True


def build_program(use_collective=USE_COLLECTIVE):
    nc = bacc.Bacc("TRN2", target_bir_lowering=False, debug=False,
                   num_devices=N_CORES)

    xt = nc.dram_tensor("xt", [E, L], BF16, kind="ExternalInput").ap()
    cost = nc.dram_tensor("cost", [D, L], BF16, kind="ExternalInput").ap()
    sint = nc.dram_tensor("sint", [D, L], BF16, kind="ExternalInput").ap()
    wqt = nc.dram_tensor("wqt", [E, 2 * D], BF16, kind="ExternalInput").ap()
    wkt = nc.dram_tensor("wkt", [E, D], BF16, kind="ExternalInput").ap()
    wvt = nc.dram_tensor("wvt", [E, D], BF16, kind="ExternalInput").ap()
    wot = nc.dram_tensor("wot", [HD, E], BF16, kind="ExternalInput").ap()
    if use_collective:
        # own-l-slice copies (per-core host slices keep the program SPMD)
        xto = nc.dram_tensor("xto", [E, 512], BF16, kind="ExternalInput").ap()
        csto = nc.dram_tensor("csto", [D, 512], BF16,
                              kind="ExternalInput").ap()
        ssto = nc.dram_tensor("ssto", [D, 512], BF16,
                              kind="ExternalInput").ap()
    out = nc.dram_tensor("out", [4 * P, E], F32, kind="ExternalOutput").ap()

    with tile.TileContext(nc) as tc, contextlib.ExitStack() as stack:
        with tc.tile_pool(name="res", bufs=1) as res:
            kT = [res.tile([P, L], BF16, tag=f"kT{i}", name=f"kT{i}")
                  for i in range(2)]
            v_bf = [res.tile([P, D], BF16, tag=f"v{i}", name=f"v{i}")
                    for i in range(LT)]
            qT = [res.tile([P, L], BF16, tag=f"qT{i}", name=f"qT{i}")
                  for i in range(4)]
            ones128 = res.tile([P, P], BF16, tag="ones128", name="ones128")
            nc.vector.memset(ones128[:], 1.0)

            if use_collective:
                dr = stack.enter_context(
                    tc.tile_pool(name="ccdram", bufs=1, space="DRAM"))
                kv_in = dr.tile([4 * P, 512], BF16, name="kv_in")
                kv_out = dr.tile([16 * P, 512], BF16, name="kv_out")

            # ---------------- Phase 1: projections + RoPE ----------------
            with tc.tile_pool(name="proj", bufs=1) as proj, \
                 tc.tile_pool(name="proj_ps", space="PSUM", bufs=1) as pps:
                wk = [proj.tile([P, D], BF16, tag=f"wk{e}", name=f"wk{e}")
                      for e in range(EC)]
                wv = [proj.tile([P, D], BF16, tag=f"wv{e}", name=f"wv{e}")
                      for e in range(EC)]
                wq = [proj.tile([P, 2 * D], BF16, tag=f"wq{e}",
                                name=f"wq{e}") for e in range(EC)]
                xf = [proj.tile([P, L], BF16, tag=f"xf{e}", name=f"xf{e}")
                      for e in range(EC)]

                def _rope(p0, p1, csk, out0, out1, tag):
                    ta = proj.tile([P, 512], F32, tag="ropetmp", bufs=4,
                                   name=f"ta{tag}")
                    tb = proj.tile([P, 512], F32, tag="ropetmp", bufs=4,
                                   name=f"tb{tag}")
                    nc.vector.tensor_tensor(ta[:], p0[:],
                                            csk[("cos", 0)][:], OP.mult)
                    nc.vector.tensor_tensor(tb[:], p1[:],
                                            csk[("sin", 0)][:], OP.mult)
                    nc.vector.tensor_tensor(out0, ta[:], tb[:], OP.subtract)
                    tc2 = proj.tile([P, 512], F32, tag="ropetmp", bufs=4,
                                    name=f"tc{tag}")
                    td = proj.tile([P, 512], F32, tag="ropetmp", bufs=4,
                                   name=f"td{tag}")
                    nc.vector.tensor_tensor(tc2[:], p1[:],
                                            csk[("cos", 1)][:], OP.mult)
                    nc.vector.tensor_tensor(td[:], p0[:],
                                            csk[("sin", 1)][:], OP.mult)
                    nc.vector.tensor_tensor(out1, tc2[:], td[:], OP.add)

                def load_csk(srcs, lc, store, tagp):
                    # srcs: (cos_dram, sin_dram); slices [half*P:(half+1)*P]
                    sl = slice(lc * 512, (lc + 1) * 512)
                    for nm, srcd in (("cos", srcs[0]), ("sin", srcs[1])):
                        for half in range(2):
                            t = proj.tile([P, 512], BF16, tag="csk", bufs=20,
                                          name=f"{tagp}{nm}{half}_{lc}")
                            nc.sync.dma_start(
                                out=t[:],
                                in_=srcd[half * P:(half + 1) * P, sl])
                            store[(nm, half)] = t

                if use_collective:
                    xo = [proj.tile([P, 512], BF16, tag=f"xo{e}",
                                    name=f"xo{e}") for e in range(EC)]
                    # K-path DMAs first: wk + own x + own cos/sin
                    for e in range(EC):
                        nc.sync.dma_start(out=wk[e][:],
                                          in_=wkt[e * P:(e + 1) * P, :])
                        nc.sync.dma_start(out=xo[e][:],
                                          in_=xto[e * P:(e + 1) * P, :])
                    cso = {}
                    load_csk((csto, ssto), 0, cso, "o")
                    for e in range(EC):
                        nc.sync.dma_start(out=wv[e][:],
                                          in_=wvt[e * P:(e + 1) * P, :])

                    # K projection for own l-slice + RoPE
                    pk0 = pps.tile([P, 512], F32, tag="pk", bufs=2,
                                   name="pk0")
                    pk1 = pps.tile([P, 512], F32, tag="pk", bufs=2,
                                   name="pk1")
                    for e in range(EC):
                        st, sp = (e == 0), (e == EC - 1)
                        nc.tensor.matmul(pk0[:], wk[e][:, 0:P], xo[e][:],
                                         start=st, stop=sp)
                        nc.tensor.matmul(pk1[:], wk[e][:, P:2 * P], xo[e][:],
                                         start=st, stop=sp)
                    kTl = [proj.tile([P, 512], BF16, tag=f"kTl{i}",
                                     name=f"kTl{i}") for i in range(2)]
                    _rope(pk0, pk1, cso, kTl[0][:], kTl[1][:], "k")
                    nc.sync.dma_start(out=kv_in[0:P, :], in_=kTl[0][:])
                    nc.sync.dma_start(out=kv_in[P:2 * P, :], in_=kTl[1][:])

                    # V projection for own l-slice
                    v_loc = [proj.tile([P, D], BF16, tag=f"vl{i}",
                                       name=f"vl{i}") for i in range(4)]
                    for lt4 in range(4):
                        pv = pps.tile([P, D], F32, tag="pv", bufs=2,
                                      name=f"pv{lt4}")
                        for e in range(EC):
                            nc.tensor.matmul(
                                pv[:], xo[e][:, lt4 * P:(lt4 + 1) * P],
                                wv[e][:], start=(e == 0), stop=(e == EC - 1))
                        nc.scalar.copy(v_loc[lt4][:], pv[:])
                        nc.sync.dma_start(
                            out=kv_in[(2 + lt4 // 2) * P:(3 + lt4 // 2) * P,
                                      (lt4 % 2) * D:(lt4 % 2 + 1) * D],
                            in_=v_loc[lt4][:])

                    nc.gpsimd.collective_compute(
                        "AllGather",
                        mybir.AluOpType.bypass,
                        replica_groups=[[0, 1, 2, 3], [4, 5, 6, 7]],
                        ins=[kv_in[:].opt()],
                        outs=[kv_out[:].opt()],
                    )

                # Q weights + full x^T, interleaved per e-chunk so the Q
                # projection can start as soon as the first chunks arrive.
                csf = [dict() for _ in range(4)]
                for e in range(EC):
                    nc.sync.dma_start(out=wq[e][:],
                                      in_=wqt[e * P:(e + 1) * P, :])
                    nc.sync.dma_start(out=xf[e][:, 0:512],
                                      in_=xt[e * P:(e + 1) * P, 0:512])
                load_csk((cost, sint), 0, csf[0], "f")
                for lc in range(1, 4):
                    sl = slice(lc * 512, (lc + 1) * 512)
                    for e in range(EC):
                        nc.sync.dma_start(out=xf[e][:, sl],
                                          in_=xt[e * P:(e + 1) * P, sl])
                    load_csk((cost, sint), lc, csf[lc], "f")

                if not use_collective:
                    # replicated K projection over the full sequence
                    for lc in range(4):
                        sl = slice(lc * 512, (lc + 1) * 512)
                        pk0 = pps.tile([P, 512], F32, tag="pk", bufs=2,
                                       name=f"pk0_{lc}")
                        pk1 = pps.tile([P, 512], F32, tag="pk", bufs=2,
                                       name=f"pk1_{lc}")
                        for e in range(EC):
                            st, sp = (e == 0), (e == EC - 1)
                            xs = xf[e][:, sl]
                            nc.tensor.matmul(pk0[:], wk[e][:, 0:P], xs,
                                             start=st, stop=sp)
                            nc.tensor.matmul(pk1[:], wk[e][:, P:2 * P], xs,
                                             start=st, stop=sp)
                        _rope(pk0, pk1, csf[lc], kT[0][:, sl], kT[1][:, sl],
                              f"k{lc}")
                    for lt in range(LT):
                        pv = pps.tile([P, D], F32, tag="pv", bufs=2,
                                      name=f"pv{lt}")
                        for e in range(EC):
                            nc.tensor.matmul(
                                pv[:], xf[e][:, lt * P:(lt + 1) * P],
                                wv[e][:], start=(e == 0), stop=(e == EC - 1))
                        nc.scalar.copy(v_bf[lt][:], pv[:])

                # Q projection + RoPE, per l-chunk
                for lc in range(4):
                    sl = slice(lc * 512, (lc + 1) * 512)
                    pq = [pps.tile([P, 512], F32, tag=f"pq{j}", bufs=1,
                                   name=f"pq{lc}_{j}") for j in range(4)]
                    for e in range(EC):
                        st, sp = (e == 0), (e == EC - 1)
                        xs = xf[e][:, sl]
                        for j in range(4):
                            nc.tensor.matmul(pq[j][:],
                                             wq[e][:, j * P:(j + 1) * P],
                                             xs, start=st, stop=sp)
                    _rope(pq[0], pq[1], csf[lc], qT[0][:, sl], qT[1][:, sl],
                          f"q0{lc}")
                    _rope(pq[2], pq[3], csf[lc], qT[2][:, sl], qT[3][:, sl],
                          f"q1{lc}")

                if use_collective:
                    # unpack the gathered K^T / V
                    for r in range(4):
                        base = r * 4 * P
                        sl = slice(r * 512, (r + 1) * 512)
                        nc.sync.dma_start(out=kT[0][:, sl],
                                          in_=kv_out[base:base + P, :])
                        nc.sync.dma_start(out=kT[1][:, sl],
                                          in_=kv_out[base + P:base + 2 * P, :])
                        for i in range(4):
                            rows = slice(base + (2 + i // 2) * P,
                                         base + (3 + i // 2) * P)
                            cols = slice((i % 2) * D, (i % 2 + 1) * D)
                            nc.sync.dma_start(out=v_bf[4 * r + i][:],
                                              in_=kv_out[rows, cols])

            # ------------- Phase 2: attention + o_proj -------------
            with tc.tile_pool(name="att", bufs=1) as att, \
                 tc.tile_pool(name="att_ps", space="PSUM", bufs=1) as aps:
                G = [[att.tile([P, L], BF16, tag=f"G{hh}{dt}",
                               name=f"G{hh}{dt}") for dt in range(2)]
                     for hh in range(2)]
                wo = [att.tile([P, E], BF16, tag=f"wo{i}", name=f"wo{i}")
                      for i in range(EC)]
                for i in range(EC):
                    nc.sync.dma_start(out=wo[i][:],
                                      in_=wot[i * P:(i + 1) * P, :])

                backlog = []

                def pop_units(n):
                    for _ in range(n):
                        if backlog:
                            backlog.pop(0)()

                def make_po_units(hh, lqc, pt_t, acc):
                    """Deferred PV chains + rowsum reduce + normalize."""
                    cell = {}

                    def po_unit(dt, lk):
                        def u():
                            if lk == 0:
                                cell[dt] = aps.tile(
                                    [P, 512], F32, tag="po", bufs=2,
                                    name=f"po{hh}_{lqc}_{dt}")
                            nc.tensor.matmul(
                                cell[dt][:],
                                v_bf[lk][:, dt * P:(dt + 1) * P],
                                pt_t[lk][:],
                                start=(lk == 0), stop=(lk == LT - 1))
                        return u

                    def fin():
                        prb = aps.tile([P, 512], F32, tag="prb", bufs=1,
                                       name=f"prb{hh}_{lqc}")
                        nc.tensor.matmul(prb[:], ones128[:], acc[:],
                                         start=True, stop=True)
                        rb = att.tile([P, 512], F32, tag="rb", bufs=2,
                                      name=f"rb{hh}_{lqc}")
                        nc.vector.reciprocal_approx_fast(out=rb[:],
                                                         in_=prb[:])
                        rb_wu = rb.rearrange("p (u w) -> p w u", w=8)
                        for dt in range(2):
                            g_dst = G[hh][dt].rearrange(
                                "p (w r) -> p w r",
                                w=8)[:, :, 64 * lqc:64 * lqc + 64]
                            nc.vector.tensor_tensor(
                                g_dst,
                                cell[dt].rearrange("p (u w) -> p w u", w=8),
                                rb_wu, OP.mult)
                    units = [po_unit(dt, lk) for dt in range(2)
                             for lk in range(LT)]
                    units.append(fin)
                    return units

                def make_oproj_units(a_idx):
                    units = []
                    for rh in range(2):
                        rt = a_idx * 2 + rh
                        for eg in range(4):
                            esl = slice(eg * 512, (eg + 1) * 512)
                            cell = {}

                            def mm_unit(m, rt=rt, rh=rh, eg=eg, esl=esl,
                                        cell=cell):
                                def u():
                                    if m == 0:
                                        cell["py"] = aps.tile(
                                            [P, 512], F32, tag="py", bufs=2,
                                            name=f"py{rt}_{eg}")
                                    lhsT = G[a_idx][m % 2][
                                        :, (m // 2) * 256 + rh * P:
                                           (m // 2) * 256 + rh * P + P]
                                    nc.tensor.matmul(cell["py"][:], lhsT,
                                                     wo[m][:, esl],
                                                     start=(m == 0),
                                                     stop=(m == EC - 1))
                                    if m == EC - 1:
                                        ysb = att.tile(
                                            [P, 512], F32, tag="ysb",
                                            bufs=3, name=f"ysb{rt}_{eg}")
                                        eng = (nc.scalar.copy if eg % 2
                                               else nc.vector.tensor_copy)
                                        eng(ysb[:], cell["py"][:])
                                        nc.sync.dma_start(
                                            out=out[rt * P:(rt + 1) * P,
                                                    esl],
                                            in_=ysb[:])
                                return u
                            units.extend(mm_unit(m) for m in range(EC))
                    return units

                for hh in range(2):
                    qh0, qh1 = qT[2 * hh], qT[2 * hh + 1]
                    for lqc in range(4):
                        qsl = slice(lqc * 512, (lqc + 1) * 512)
                        pt_t = [att.tile([P, 512], BF16, tag=f"pt{i}",
                                         bufs=4, name=f"pt{hh}_{lqc}_{i}")
                                for i in range(LT)]
                        l1 = [att.tile([P, 512], BF16, tag=f"tl1_{i}",
                                       bufs=1, name=f"l1_{hh}_{lqc}_{i}")
                              for i in range(8)]
                        l2 = [att.tile([P, 512], BF16, tag=f"tl2_{i}",
                                       bufs=1, name=f"l2_{hh}_{lqc}_{i}")
                              for i in range(4)]
                        l3 = [att.tile([P, 512], BF16, tag=f"tl3_{i}",
                                       bufs=1, name=f"l3_{hh}_{lqc}_{i}")
                              for i in range(2)]
                        acc = att.tile([P, 512], BF16, tag="tacc", bufs=4,
                                       name=f"acc{hh}_{lqc}")
                        for lk in range(LT):
                            ps = aps.tile([P, 512], F32, tag="ps", bufs=3,
                                          name=f"ps{hh}_{lqc}_{lk}")
                            nc.tensor.matmul(ps[:],
                                             kT[0][:, lk * P:(lk + 1) * P],
                                             qh0[:, qsl],
                                             start=True, stop=False)
                            nc.tensor.matmul(ps[:],
                                             kT[1][:, lk * P:(lk + 1) * P],
                                             qh1[:, qsl],
                                             start=False, stop=True)
                            nc.scalar.activation(pt_t[lk][:], ps[:], AF.Exp,
                                                 scale=float(SCALING))
                            if lk % 2 == 1:
                                nc.vector.tensor_tensor(
                                    l1[lk // 2][:], pt_t[lk - 1][:],
                                    pt_t[lk][:], OP.add)
                            pop_units(2)
                        for i in range(4):
                            nc.vector.tensor_tensor(l2[i][:], l1[2 * i][:],
                                                    l1[2 * i + 1][:], OP.add)
                        for i in range(2):
                            nc.vector.tensor_tensor(l3[i][:], l2[2 * i][:],
                                                    l2[2 * i + 1][:], OP.add)
                        nc.vector.tensor_tensor(acc[:], l3[0][:], l3[1][:],
                                                OP.add)
                        backlog.extend(make_po_units(hh, lqc, pt_t, acc))
                        if lqc == 3:
                            backlog.extend(make_oproj_units(hh))
                # flush remaining deferred PE work
                pop_units(len(backlog))

    nc.compile()
    return nc


_NC = None


def _get_nc():
    global _NC
    if _NC is None:
        _NC = build_program()
    return _NC


def _bf16(a):
    return np.ascontiguousarray(a).astype(ml_dtypes.bfloat16)


def make_in_maps(hidden_states, cos, sin, Wq, Wk, Wv, Wo):
    hs = np.asarray(hidden_states, np.float32)
    xt = [_bf16(hs[b].T) for b in range(B)]
    cost = _bf16(np.asarray(cos, np.float32).T)
    sint = _bf16(np.asarray(sin, np.float32).T)
    wqt = np.ascontiguousarray(np.asarray(Wq, np.float32).T)
    wkt = _bf16(np.asarray(Wk, np.float32).T)
    wvt = _bf16(np.asarray(Wv, np.float32).T)
    wot = _bf16(np.asarray(Wo, np.float32).T)
    in_maps = []
    for c in range(N_CORES):
        b, ql = c // 4, c % 4
        sl = slice(ql * 512, (ql + 1) * 512)
        m = {
            "xt": xt[b],
            "cost": cost,
            "sint": sint,
            "wqt": _bf16(wqt[:, sl]),
            "wkt": wkt,
            "wvt": wvt,
            "wot": wot,
        }
        if USE_COLLECTIVE:
            m["xto"] = np.ascontiguousarray(xt[b][:, sl])
            m["csto"] = np.ascontiguousarray(cost[:, sl])
            m["ssto"] = np.ascontiguousarray(sint[:, sl])
        in_maps.append(m)
    return in_maps


def assemble(results):
    y = np.empty((B, L, E), np.float32)
    for c in range(N_CORES):
        b, ql = c // 4, c % 4
        y[b, ql * 512:(ql + 1) * 512, :] = results[c]["out"]
    return y


def kernel(hidden_states, attention_mask, cos, sin, Wq, Wk, Wv, Wo):
    # attention_mask is additive and all-zero per the problem spec; it is
    # accepted for signature compatibility but not shipped to the device.
    nc = _get_nc()
    in_maps = make_in_maps(hidden_states, cos, sin, Wq, Wk, Wv, Wo)
    res = run_bass_kernel_spmd(nc, in_maps, core_ids=list(range(N_CORES)))
    return assemble(res.results)


# revision 6
# speedup vs baseline: 1.0493x; 1.0142x over previous
"""ColPali MQA attention block on 8 Trainium2 NeuronCores.

The reference contains the ColPali reshape quirk: the attention output
[B, H, L, 1, D] is reshaped row-major straight to [B, L, H*D], which mixes
heads and positions.  Output row l' therefore depends ONLY on head
h = l'//256, gathering positions (l'%256)*8 + j for j in 0..7.

Sharding: core c -> batch b=c//4 and heads {h0, h0+1} with h0=2*(c%4).
Each core computes the K/V projection for its batch (replicated inside the
4-core batch group -- a cross-core AllGather was measured at ~100us
effective latency in this environment, worse than the 42us of replicated
PE work), Q projection + attention for its 2 heads, and o_proj for output
rows [512*ql, 512*ql+512).  Per-core outputs are disjoint slices -> no
cross-core communication.

All inputs are pre-converted to bf16 on the host and DMA'd straight into
their SBUF layouts (no on-device staging casts); Wk/Wv ship concatenated
so every DMA has >=1KB contiguous lines.  Phase 1 runs lc-chunk-major
(K -> V -> Q per 512-column chunk) so the PE never idles long enough for
the HAM clock gate to re-throttle.

The softmax row-sum is a pairwise bf16 add-tree on the vector engine plus
one all-ones matmul per block for the cross-partition reduction (instead
of 16 PE matmuls); 1/rowsum uses the fast approximate DVE reciprocal
(~18 bits, plenty against the 2e-2 gate).  Phase 2 is software-pipelined:
the scores+exp stream for attention block N is interleaved (2 matmuls per
score-pair) with a FIFO backlog of deferred PE work (PV accumulation of
block N-1, rowsum reduce, o_proj chains), so the PE never waits on the
scalar engine's exp drain.
"""

import numpy as np
import ml_dtypes

import concourse.mybir as mybir
import concourse.tile as tile
from concourse import bacc
from concourse.bass_utils import run_bass_kernel_spmd

F32 = mybir.dt.float32
BF16 = mybir.dt.bfloat16
AF = mybir.ActivationFunctionType
OP = mybir.AluOpType

B, L, H, D, E = 2, 2048, 8, 256, 2048
HD = H * D  # 2048
P = 128
EC = E // P  # 16 e-chunks
LT = L // P  # 16 l-tiles
SCALING = D ** -0.5  # 1/16
N_CORES = 8


def build_program():
    nc = bacc.Bacc("TRN2", target_bir_lowering=False, debug=False,
                   num_devices=N_CORES)

    xt = nc.dram_tensor("xt", [E, L], BF16, kind="ExternalInput").ap()
    cost = nc.dram_tensor("cost", [D, L], BF16, kind="ExternalInput").ap()
    sint = nc.dram_tensor("sint", [D, L], BF16, kind="ExternalInput").ap()
    wqt = nc.dram_tensor("wqt", [E, 2 * D], BF16, kind="ExternalInput").ap()
    # Wk and Wv concatenated on the output dim: [E, 2*D]
    wkvt = nc.dram_tensor("wkvt", [E, 2 * D], BF16,
                          kind="ExternalInput").ap()
    wot = nc.dram_tensor("wot", [HD, E], BF16, kind="ExternalInput").ap()
    out = nc.dram_tensor("out", [4 * P, E], F32, kind="ExternalOutput").ap()

    with tile.TileContext(nc) as tc:
        with tc.tile_pool(name="res", bufs=1) as res:
            kT = [res.tile([P, L], BF16, tag=f"kT{i}", name=f"kT{i}")
                  for i in range(2)]
            v_bf = [res.tile([P, D], BF16, tag=f"v{i}", name=f"v{i}")
                    for i in range(LT)]
            qT = [res.tile([P, L], BF16, tag=f"qT{i}", name=f"qT{i}")
                  for i in range(4)]
            ones128 = res.tile([P, P], BF16, tag="ones128", name="ones128")
            nc.vector.memset(ones128[:], 1.0)

            # ---------------- Phase 1: projections + RoPE ----------------
            with tc.tile_pool(name="proj", bufs=1) as proj, \
                 tc.tile_pool(name="proj_ps", space="PSUM", bufs=1) as pps:
                wkv = [proj.tile([P, 2 * D], BF16, tag=f"wkv{e}",
                                 name=f"wkv{e}") for e in range(EC)]
                wq = [proj.tile([P, 2 * D], BF16, tag=f"wq{e}",
                                name=f"wq{e}") for e in range(EC)]
                xf = [proj.tile([P, L], BF16, tag=f"xf{e}", name=f"xf{e}")
                      for e in range(EC)]
                # cos/sin tiles per (name, half, lc-pair): [128, 1024]
                csk = {}

                def load_csk(pair):
                    sl = slice(pair * 1024, (pair + 1) * 1024)
                    for nm, srcd in (("cos", cost), ("sin", sint)):
                        for half in range(2):
                            t = proj.tile([P, 1024], BF16, tag="csk",
                                          bufs=8, name=f"cs{nm}{half}{pair}")
                            nc.sync.dma_start(
                                out=t[:],
                                in_=srcd[half * P:(half + 1) * P, sl])
                            csk[(nm, half, pair)] = t

                # first-needed DMAs first; 1-2KB contiguous lines throughout
                for e in range(EC):
                    nc.sync.dma_start(out=wkv[e][:],
                                      in_=wkvt[e * P:(e + 1) * P, :])
                    nc.sync.dma_start(out=xf[e][:, 0:1024],
                                      in_=xt[e * P:(e + 1) * P, 0:1024])
                load_csk(0)
                for e in range(EC):
                    nc.sync.dma_start(out=wq[e][:],
                                      in_=wqt[e * P:(e + 1) * P, :])
                for e in range(EC):
                    nc.sync.dma_start(out=xf[e][:, 1024:2048],
                                      in_=xt[e * P:(e + 1) * P, 1024:2048])
                load_csk(1)

                def _rope(p0, p1, cs, out0, out1, tag):
                    ta = proj.tile([P, 512], F32, tag="ropetmp", bufs=4,
                                   name=f"ta{tag}")
                    tb = proj.tile([P, 512], F32, tag="ropetmp", bufs=4,
                                   name=f"tb{tag}")
                    nc.vector.tensor_tensor(ta[:], p0[:], cs[0], OP.mult)
                    nc.vector.tensor_tensor(tb[:], p1[:], cs[1], OP.mult)
                    nc.vector.tensor_tensor(out0, ta[:], tb[:], OP.subtract)
                    tc2 = proj.tile([P, 512], F32, tag="ropetmp", bufs=4,
                                    name=f"tc{tag}")
                    td = proj.tile([P, 512], F32, tag="ropetmp", bufs=4,
                                   name=f"td{tag}")
                    nc.vector.tensor_tensor(tc2[:], p1[:], cs[2], OP.mult)
                    nc.vector.tensor_tensor(td[:], p0[:], cs[3], OP.mult)
                    nc.vector.tensor_tensor(out1, tc2[:], td[:], OP.add)

                # K -> V -> Q per 512-column l-chunk: PE stays dense, each
                # chunk's inputs arrive while the previous chunk computes.
                for lc in range(4):
                    sl = slice(lc * 512, (lc + 1) * 512)
                    pair, ph = lc // 2, lc % 2
                    psl = slice(ph * 512, (ph + 1) * 512)
                    cs = [csk[("cos", 0, pair)][:, psl],
                          csk[("sin", 0, pair)][:, psl],
                          csk[("cos", 1, pair)][:, psl],
                          csk[("sin", 1, pair)][:, psl]]

                    pk0 = pps.tile([P, 512], F32, tag="pk", bufs=2,
                                   name=f"pk0_{lc}")
                    pk1 = pps.tile([P, 512], F32, tag="pk", bufs=2,
                                   name=f"pk1_{lc}")
                    for e in range(EC):
                        st, sp = (e == 0), (e == EC - 1)
                        xs = xf[e][:, sl]
                        nc.tensor.matmul(pk0[:], wkv[e][:, 0:P], xs,
                                         start=st, stop=sp)
                        nc.tensor.matmul(pk1[:], wkv[e][:, P:2 * P], xs,
                                         start=st, stop=sp)
                    _rope(pk0, pk1, cs, kT[0][:, sl], kT[1][:, sl],
                          f"k{lc}")

                    for lt in range(4 * lc, 4 * lc + 4):
                        pv = pps.tile([P, D], F32, tag="pv", bufs=2,
                                      name=f"pv{lt}")
                        for e in range(EC):
                            nc.tensor.matmul(
                                pv[:], xf[e][:, lt * P:(lt + 1) * P],
                                wkv[e][:, 2 * P:4 * P],
                                start=(e == 0), stop=(e == EC - 1))
                        nc.scalar.copy(v_bf[lt][:], pv[:])

                    pq = [pps.tile([P, 512], F32, tag=f"pq{j}", bufs=1,
                                   name=f"pq{lc}_{j}") for j in range(4)]
                    for e in range(EC):
                        st, sp = (e == 0), (e == EC - 1)
                        xs = xf[e][:, sl]
                        for j in range(4):
                            nc.tensor.matmul(pq[j][:],
                                             wq[e][:, j * P:(j + 1) * P],
                                             xs, start=st, stop=sp)
                    _rope(pq[0], pq[1], cs, qT[0][:, sl], qT[1][:, sl],
                          f"q0{lc}")
                    _rope(pq[2], pq[3], cs, qT[2][:, sl], qT[3][:, sl],
                          f"q1{lc}")

            # ------------- Phase 2: attention + o_proj -------------
            with tc.tile_pool(name="att", bufs=1) as att, \
                 tc.tile_pool(name="att_ps", space="PSUM", bufs=1) as aps:
                G = [[att.tile([P, L], BF16, tag=f"G{hh}{dt}",
                               name=f"G{hh}{dt}") for dt in range(2)]
                     for hh in range(2)]
                wo = [att.tile([P, E], BF16, tag=f"wo{i}", name=f"wo{i}")
                      for i in range(EC)]
                for i in range(EC):
                    nc.sync.dma_start(out=wo[i][:],
                                      in_=wot[i * P:(i + 1) * P, :])

                backlog = []

                def pop_units(n):
                    for _ in range(n):
                        if backlog:
                            backlog.pop(0)()

                def make_po_units(hh, lqc, pt_t, acc):
                    """Deferred PV chains + rowsum reduce + normalize."""
                    cell = {}

                    def po_unit(dt, lk):
                        def u():
                            if lk == 0:
                                cell[dt] = aps.tile(
                                    [P, 512], F32, tag="po", bufs=2,
                                    name=f"po{hh}_{lqc}_{dt}")
                            nc.tensor.matmul(
                                cell[dt][:],
                                v_bf[lk][:, dt * P:(dt + 1) * P],
                                pt_t[lk][:],
                                start=(lk == 0), stop=(lk == LT - 1))
                        return u

                    def fin():
                        prb = aps.tile([P, 512], F32, tag="prb", bufs=1,
                                       name=f"prb{hh}_{lqc}")
                        nc.tensor.matmul(prb[:], ones128[:], acc[:],
                                         start=True, stop=True)
                        rb = att.tile([P, 512], F32, tag="rb", bufs=2,
                                      name=f"rb{hh}_{lqc}")
                        nc.vector.reciprocal_approx_fast(out=rb[:],
                                                         in_=prb[:])
                        rb_wu = rb.rearrange("p (u w) -> p w u", w=8)
                        for dt in range(2):
                            g_dst = G[hh][dt].rearrange(
                                "p (w r) -> p w r",
                                w=8)[:, :, 64 * lqc:64 * lqc + 64]
                            nc.vector.tensor_tensor(
                                g_dst,
                                cell[dt].rearrange("p (u w) -> p w u", w=8),
                                rb_wu, OP.mult)
                    units = [po_unit(dt, lk) for dt in range(2)
                             for lk in range(LT)]
                    units.append(fin)
                    return units

                def make_oproj_units(a_idx):
                    units = []
                    for rh in range(2):
                        rt = a_idx * 2 + rh
                        for eg in range(4):
                            esl = slice(eg * 512, (eg + 1) * 512)
                            cell = {}

                            def mm_unit(m, rt=rt, rh=rh, eg=eg, esl=esl,
                                        cell=cell):
                                def u():
                                    if m == 0:
                                        cell["py"] = aps.tile(
                                            [P, 512], F32, tag="py", bufs=2,
                                            name=f"py{rt}_{eg}")
                                    lhsT = G[a_idx][m % 2][
                                        :, (m // 2) * 256 + rh * P:
                                           (m // 2) * 256 + rh * P + P]
                                    nc.tensor.matmul(cell["py"][:], lhsT,
                                                     wo[m][:, esl],
                                                     start=(m == 0),
                                                     stop=(m == EC - 1))
                                    if m == EC - 1:
                                        ysb = att.tile(
                                            [P, 512], F32, tag="ysb",
                                            bufs=3, name=f"ysb{rt}_{eg}")
                                        eng = (nc.scalar.copy if eg % 2
                                               else nc.vector.tensor_copy)
                                        eng(ysb[:], cell["py"][:])
                                        nc.sync.dma_start(
                                            out=out[rt * P:(rt + 1) * P,
                                                    esl],
                                            in_=ysb[:])
                                return u
                            units.extend(mm_unit(m) for m in range(EC))
                    return units

                for hh in range(2):
                    qh0, qh1 = qT[2 * hh], qT[2 * hh + 1]
                    for lqc in range(4):
                        qsl = slice(lqc * 512, (lqc + 1) * 512)
                        pt_t = [att.tile([P, 512], BF16, tag=f"pt{i}",
                                         bufs=4, name=f"pt{hh}_{lqc}_{i}")
                                for i in range(LT)]
                        l1 = [att.tile([P, 512], BF16, tag=f"tl1_{i}",
                                       bufs=1, name=f"l1_{hh}_{lqc}_{i}")
                              for i in range(8)]
                        l2 = [att.tile([P, 512], BF16, tag=f"tl2_{i}",
                                       bufs=1, name=f"l2_{hh}_{lqc}_{i}")
                              for i in range(4)]
                        l3 = [att.tile([P, 512], BF16, tag=f"tl3_{i}",
                                       bufs=1, name=f"l3_{hh}_{lqc}_{i}")
                              for i in range(2)]
                        acc = att.tile([P, 512], BF16, tag="tacc", bufs=4,
                                       name=f"acc{hh}_{lqc}")
                        for lk in range(LT):
                            ps = aps.tile([P, 512], F32, tag="ps", bufs=3,
                                          name=f"ps{hh}_{lqc}_{lk}")
                            nc.tensor.matmul(ps[:],
                                             kT[0][:, lk * P:(lk + 1) * P],
                                             qh0[:, qsl],
                                             start=True, stop=False)
                            nc.tensor.matmul(ps[:],
                                             kT[1][:, lk * P:(lk + 1) * P],
                                             qh1[:, qsl],
                                             start=False, stop=True)
                            nc.scalar.activation(pt_t[lk][:], ps[:], AF.Exp,
                                                 scale=float(SCALING))
                            if lk % 2 == 1:
                                nc.vector.tensor_tensor(
                                    l1[lk // 2][:], pt_t[lk - 1][:],
                                    pt_t[lk][:], OP.add)
                            pop_units(2)
                        for i in range(4):
                            nc.vector.tensor_tensor(l2[i][:], l1[2 * i][:],
                                                    l1[2 * i + 1][:], OP.add)
                        for i in range(2):
                            nc.vector.tensor_tensor(l3[i][:], l2[2 * i][:],
                                                    l2[2 * i + 1][:], OP.add)
                        nc.vector.tensor_tensor(acc[:], l3[0][:], l3[1][:],
                                                OP.add)
                        backlog.extend(make_po_units(hh, lqc, pt_t, acc))
                        if lqc == 3:
                            backlog.extend(make_oproj_units(hh))
                # flush remaining deferred PE work
                pop_units(len(backlog))

    nc.compile()
    return nc


_NC = None


def _get_nc():
    global _NC
    if _NC is None:
        _NC = build_program()
    return _NC


def _bf16(a):
    return np.ascontiguousarray(a).astype(ml_dtypes.bfloat16)


def make_in_maps(hidden_states, cos, sin, Wq, Wk, Wv, Wo):
    hs = np.asarray(hidden_states, np.float32)
    xt = [_bf16(hs[b].T) for b in range(B)]
    cost = _bf16(np.asarray(cos, np.float32).T)
    sint = _bf16(np.asarray(sin, np.float32).T)
    wqt = np.ascontiguousarray(np.asarray(Wq, np.float32).T)
    wkvt = _bf16(np.concatenate(
        [np.asarray(Wk, np.float32).T, np.asarray(Wv, np.float32).T],
        axis=1))
    wot = _bf16(np.asarray(Wo, np.float32).T)
    in_maps = []
    for c in range(N_CORES):
        b, ql = c // 4, c % 4
        in_maps.append({
            "xt": xt[b],
            "cost": cost,
            "sint": sint,
            "wqt": _bf16(wqt[:, ql * 512:(ql + 1) * 512]),
            "wkvt": wkvt,
            "wot": wot,
        })
    return in_maps


def assemble(results):
    y = np.empty((B, L, E), np.float32)
    for c in range(N_CORES):
        b, ql = c // 4, c % 4
        y[b, ql * 512:(ql + 1) * 512, :] = results[c]["out"]
    return y


def kernel(hidden_states, attention_mask, cos, sin, Wq, Wk, Wv, Wo):
    # attention_mask is additive and all-zero per the problem spec; it is
    # accepted for signature compatibility but not shipped to the device.
    nc = _get_nc()
    in_maps = make_in_maps(hidden_states, cos, sin, Wq, Wk, Wv, Wo)
    res = run_bass_kernel_spmd(nc, in_maps, core_ids=list(range(N_CORES)))
    return assemble(res.results)


# revision 12
# speedup vs baseline: 1.0819x; 1.0311x over previous
"""ColPali MQA attention block on 8 Trainium2 NeuronCores.

The reference contains the ColPali reshape quirk: the attention output
[B, H, L, 1, D] is reshaped row-major straight to [B, L, H*D], which mixes
heads and positions.  Output row l' therefore depends ONLY on head
h = l'//256, gathering positions (l'%256)*8 + j for j in 0..7.

Sharding: core c -> batch b=c//4 and heads {h0, h0+1} with h0=2*(c%4).
Each core computes the K/V projection for its batch (replicated inside the
4-core batch group -- a cross-core AllGather was measured at ~100us
effective latency in this environment, worse than the 42us of replicated
PE work), Q projection + attention for its 2 heads, and o_proj for output
rows [512*ql, 512*ql+512).  Per-core outputs are disjoint slices -> no
cross-core communication.

All inputs are pre-converted to bf16 on the host and DMA'd straight into
their SBUF layouts (no on-device staging casts); Wk/Wv ship concatenated
so every DMA has >=1KB contiguous lines.  Phase 1 runs lc-chunk-major
(K -> V -> Q per 512-column chunk) so the PE never idles long enough for
the HAM clock gate to re-throttle.

The softmax row-sum is a pairwise bf16 add-tree on the vector engine plus
one all-ones matmul per block for the cross-partition reduction (instead
of 16 PE matmuls); 1/rowsum uses the fast approximate DVE reciprocal
(~18 bits, plenty against the 2e-2 gate).  Phase 2 is software-pipelined:
the scores+exp stream for attention block N is interleaved (2 matmuls per
score-pair) with a FIFO backlog of deferred PE work (PV accumulation of
block N-1, rowsum reduce, o_proj chains), so the PE never waits on the
scalar engine's exp drain.
"""

import numpy as np
import ml_dtypes

import concourse.mybir as mybir
import concourse.tile as tile
from concourse import bacc
from concourse.bass_utils import run_bass_kernel_spmd

F32 = mybir.dt.float32
BF16 = mybir.dt.bfloat16
AF = mybir.ActivationFunctionType
OP = mybir.AluOpType

B, L, H, D, E = 2, 2048, 8, 256, 2048
HD = H * D  # 2048
P = 128
EC = E // P  # 16 e-chunks
LT = L // P  # 16 l-tiles
SCALING = D ** -0.5  # 1/16
N_CORES = 8


def build_program():
    nc = bacc.Bacc("TRN2", target_bir_lowering=False, debug=False,
                   num_devices=N_CORES)

    xt = nc.dram_tensor("xt", [E, L], BF16, kind="ExternalInput").ap()
    cost = nc.dram_tensor("cost", [D, L], BF16, kind="ExternalInput").ap()
    sint = nc.dram_tensor("sint", [D, L], BF16, kind="ExternalInput").ap()
    wqt = nc.dram_tensor("wqt", [E, 2 * D], BF16, kind="ExternalInput").ap()
    # Wk and Wv concatenated on the output dim: [E, 2*D]
    wkvt = nc.dram_tensor("wkvt", [E, 2 * D], BF16,
                          kind="ExternalInput").ap()
    wot = nc.dram_tensor("wot", [HD, E], BF16, kind="ExternalInput").ap()
    out = nc.dram_tensor("out", [4 * P, E], F32, kind="ExternalOutput").ap()

    with tile.TileContext(nc) as tc:
        with tc.tile_pool(name="res", bufs=1) as res:
            kT = [res.tile([P, L], BF16, tag=f"kT{i}", name=f"kT{i}")
                  for i in range(2)]
            v_bf = [res.tile([P, D], BF16, tag=f"v{i}", name=f"v{i}")
                    for i in range(LT)]
            qT = [res.tile([P, L], BF16, tag=f"qT{i}", name=f"qT{i}")
                  for i in range(4)]
            ones128 = res.tile([P, P], BF16, tag="ones128", name="ones128")
            nc.vector.memset(ones128[:], 1.0)
            # pre-load the scalar engine's Exp table so the first real exp
            # in phase 2 doesn't pay the ACT_TABLE_LOAD (~1.3us)
            warm = res.tile([P, 8], F32, tag="warm", name="warm")
            nc.scalar.activation(warm[:], ones128[:, 0:8], AF.Exp)

            # ---------------- Phase 1: projections + RoPE ----------------
            with tc.tile_pool(name="proj", bufs=1) as proj, \
                 tc.tile_pool(name="proj_ps", space="PSUM", bufs=1) as pps:
                wkv = [proj.tile([P, 2 * D], BF16, tag=f"wkv{e}",
                                 name=f"wkv{e}") for e in range(EC)]
                wq = [proj.tile([P, 2 * D], BF16, tag=f"wq{e}",
                                name=f"wq{e}") for e in range(EC)]
                xf = [proj.tile([P, L], BF16, tag=f"xf{e}", name=f"xf{e}")
                      for e in range(EC)]
                # cos/sin tiles per (name, half, lc-pair): [128, 1024]
                csk = {}

                def load_csk(pair, nsplit=1):
                    sl = slice(pair * 1024, (pair + 1) * 1024)
                    for nm, srcd in (("cos", cost), ("sin", sint)):
                        for half in range(2):
                            t = proj.tile([P, 1024], BF16, tag="csk",
                                          bufs=8, name=f"cs{nm}{half}{pair}")
                            step = P // nsplit
                            for i in range(nsplit):
                                nc.sync.dma_start(
                                    out=t[i * step:(i + 1) * step, :],
                                    in_=srcd[half * P + i * step:
                                             half * P + (i + 1) * step, sl])
                            csk[(nm, half, pair)] = t

                def dma_split(dst, src, e, cols, n):
                    # partition-split one [128, w] transfer into n pieces so
                    # the first-needed tiles land via n parallel DMA queues
                    step = P // n
                    for i in range(n):
                        rs = slice(e * P + i * step, e * P + (i + 1) * step)
                        nc.sync.dma_start(out=dst[i * step:(i + 1) * step,
                                                  cols],
                                          in_=src[rs, cols])

                # first-needed DMAs first; 1-2KB contiguous lines
                # throughout, early chunks split across queues
                for e in range(EC):
                    n = 4 if e < 4 else (2 if e < 8 else 1)
                    dma_split(wkv[e], wkvt, e, slice(0, 2 * D), n)
                    dma_split(xf[e], xt, e, slice(0, 1024), n)
                    dma_split(wq[e], wqt, e, slice(0, 2 * D),
                              2 if e < 8 else 1)
                    if e == 7:
                        load_csk(0, nsplit=2)
                for e in range(EC):
                    nc.sync.dma_start(out=xf[e][:, 1024:2048],
                                      in_=xt[e * P:(e + 1) * P, 1024:2048])
                load_csk(1)

                def _rope(p0, p1, cs, out0, out1, tag):
                    ta = proj.tile([P, 512], F32, tag="ropetmp", bufs=4,
                                   name=f"ta{tag}")
                    tb = proj.tile([P, 512], F32, tag="ropetmp", bufs=4,
                                   name=f"tb{tag}")
                    nc.vector.tensor_tensor(ta[:], p0[:], cs[0], OP.mult)
                    nc.vector.tensor_tensor(tb[:], p1[:], cs[1], OP.mult)
                    nc.vector.tensor_tensor(out0, ta[:], tb[:], OP.subtract)
                    tc2 = proj.tile([P, 512], F32, tag="ropetmp", bufs=4,
                                    name=f"tc{tag}")
                    td = proj.tile([P, 512], F32, tag="ropetmp", bufs=4,
                                   name=f"td{tag}")
                    nc.vector.tensor_tensor(tc2[:], p1[:], cs[2], OP.mult)
                    nc.vector.tensor_tensor(td[:], p0[:], cs[3], OP.mult)
                    nc.vector.tensor_tensor(out1, tc2[:], td[:], OP.add)

                # K -> V -> Q per 512-column l-chunk: PE stays dense, each
                # chunk's inputs arrive while the previous chunk computes.
                for lc in range(4):
                    sl = slice(lc * 512, (lc + 1) * 512)
                    pair, ph = lc // 2, lc % 2
                    psl = slice(ph * 512, (ph + 1) * 512)
                    cs = [csk[("cos", 0, pair)][:, psl],
                          csk[("sin", 0, pair)][:, psl],
                          csk[("cos", 1, pair)][:, psl],
                          csk[("sin", 1, pair)][:, psl]]

                    pk0 = pps.tile([P, 512], F32, tag="pk", bufs=2,
                                   name=f"pk0_{lc}")
                    pk1 = pps.tile([P, 512], F32, tag="pk", bufs=2,
                                   name=f"pk1_{lc}")
                    for e in range(EC):
                        st, sp = (e == 0), (e == EC - 1)
                        xs = xf[e][:, sl]
                        nc.tensor.matmul(pk0[:], wkv[e][:, 0:P], xs,
                                         start=st, stop=sp)
                        nc.tensor.matmul(pk1[:], wkv[e][:, P:2 * P], xs,
                                         start=st, stop=sp)
                    _rope(pk0, pk1, cs, kT[0][:, sl], kT[1][:, sl],
                          f"k{lc}")

                    # head-A Q first: its rope retires off the vector queue
                    # while V/head-B still compute, so phase 2 (which needs
                    # kT + qT[0,1]) starts without waiting on vector.
                    pq = [pps.tile([P, 512], F32, tag=f"pq{j}", bufs=1,
                                   name=f"pq{lc}_{j}") for j in range(4)]
                    for e in range(EC):
                        st, sp = (e == 0), (e == EC - 1)
                        xs = xf[e][:, sl]
                        nc.tensor.matmul(pq[0][:], wq[e][:, 0:P], xs,
                                         start=st, stop=sp)
                        nc.tensor.matmul(pq[1][:], wq[e][:, P:2 * P], xs,
                                         start=st, stop=sp)
                    _rope(pq[0], pq[1], cs, qT[0][:, sl], qT[1][:, sl],
                          f"q0{lc}")

                    for lt in range(4 * lc, 4 * lc + 4):
                        pv = pps.tile([P, D], F32, tag="pv", bufs=2,
                                      name=f"pv{lt}")
                        for e in range(EC):
                            nc.tensor.matmul(
                                pv[:], xf[e][:, lt * P:(lt + 1) * P],
                                wkv[e][:, 2 * P:4 * P],
                                start=(e == 0), stop=(e == EC - 1))
                        nc.scalar.copy(v_bf[lt][:], pv[:])

                    for e in range(EC):
                        st, sp = (e == 0), (e == EC - 1)
                        xs = xf[e][:, sl]
                        nc.tensor.matmul(pq[2][:], wq[e][:, 2 * P:3 * P],
                                         xs, start=st, stop=sp)
                        nc.tensor.matmul(pq[3][:], wq[e][:, 3 * P:4 * P],
                                         xs, start=st, stop=sp)
                    _rope(pq[2], pq[3], cs, qT[2][:, sl], qT[3][:, sl],
                          f"q1{lc}")

            # ------------- Phase 2: attention + o_proj -------------
            with tc.tile_pool(name="att", bufs=1) as att, \
                 tc.tile_pool(name="att_ps", space="PSUM", bufs=1) as aps:
                G = [[att.tile([P, L], BF16, tag=f"G{hh}{dt}",
                               name=f"G{hh}{dt}") for dt in range(2)]
                     for hh in range(2)]
                wo = [att.tile([P, E], BF16, tag=f"wo{i}", name=f"wo{i}")
                      for i in range(EC)]
                for i in range(EC):
                    nc.sync.dma_start(out=wo[i][:],
                                      in_=wot[i * P:(i + 1) * P, :])

                backlog = []

                def pop_units(n):
                    for _ in range(n):
                        if backlog:
                            backlog.pop(0)()

                def make_po_units(hh, lqc, pt_t, acc):
                    """Deferred PV chains + rowsum reduce + normalize."""
                    cell = {}

                    def po_unit(dt, lk):
                        def u():
                            if lk == 0:
                                cell[dt] = aps.tile(
                                    [P, 512], F32, tag="po", bufs=2,
                                    name=f"po{hh}_{lqc}_{dt}")
                            nc.tensor.matmul(
                                cell[dt][:],
                                v_bf[lk][:, dt * P:(dt + 1) * P],
                                pt_t[lk][:],
                                start=(lk == 0), stop=(lk == LT - 1))
                        return u

                    def fin():
                        prb = aps.tile([P, 512], F32, tag="prb", bufs=1,
                                       name=f"prb{hh}_{lqc}")
                        nc.tensor.matmul(prb[:], ones128[:], acc[:],
                                         start=True, stop=True)
                        rb = att.tile([P, 512], F32, tag="rb", bufs=2,
                                      name=f"rb{hh}_{lqc}")
                        nc.vector.reciprocal_approx_fast(out=rb[:],
                                                         in_=prb[:])
                        rb_wu = rb.rearrange("p (u w) -> p w u", w=8)
                        for dt in range(2):
                            g_dst = G[hh][dt].rearrange(
                                "p (w r) -> p w r",
                                w=8)[:, :, 64 * lqc:64 * lqc + 64]
                            nc.vector.tensor_tensor(
                                g_dst,
                                cell[dt].rearrange("p (u w) -> p w u", w=8),
                                rb_wu, OP.mult)
                    units = [po_unit(dt, lk) for dt in range(2)
                             for lk in range(LT)]
                    units.append(fin)
                    return units

                def make_oproj_units(a_idx):
                    units = []
                    for rh in range(2):
                        rt = a_idx * 2 + rh
                        for eg in range(4):
                            esl = slice(eg * 512, (eg + 1) * 512)
                            cell = {}

                            def mm_unit(m, rt=rt, rh=rh, eg=eg, esl=esl,
                                        cell=cell):
                                def u():
                                    if m == 0:
                                        cell["py"] = aps.tile(
                                            [P, 512], F32, tag="py", bufs=2,
                                            name=f"py{rt}_{eg}")
                                    lhsT = G[a_idx][m % 2][
                                        :, (m // 2) * 256 + rh * P:
                                           (m // 2) * 256 + rh * P + P]
                                    nc.tensor.matmul(cell["py"][:], lhsT,
                                                     wo[m][:, esl],
                                                     start=(m == 0),
                                                     stop=(m == EC - 1))
                                    if m == EC - 1:
                                        ysb = att.tile(
                                            [P, 512], F32, tag="ysb",
                                            bufs=3, name=f"ysb{rt}_{eg}")
                                        eng = (nc.scalar.copy if eg % 2
                                               else nc.vector.tensor_copy)
                                        eng(ysb[:], cell["py"][:])
                                        # 2-way split -> 2 parallel queues
                                        for i in range(2):
                                            rsl = slice(
                                                rt * P + i * 64,
                                                rt * P + (i + 1) * 64)
                                            nc.sync.dma_start(
                                                out=out[rsl, esl],
                                                in_=ysb[i * 64:(i + 1) * 64,
                                                        :])
                                return u
                            units.extend(mm_unit(m) for m in range(EC))
                    return units

                for hh in range(2):
                    qh0, qh1 = qT[2 * hh], qT[2 * hh + 1]
                    for lqc in range(4):
                        qsl = slice(lqc * 512, (lqc + 1) * 512)
                        pt_t = [att.tile([P, 512], BF16, tag=f"pt{i}",
                                         bufs=4, name=f"pt{hh}_{lqc}_{i}")
                                for i in range(LT)]
                        l1 = [att.tile([P, 512], BF16, tag=f"tl1_{i}",
                                       bufs=1, name=f"l1_{hh}_{lqc}_{i}")
                              for i in range(8)]
                        l2 = [att.tile([P, 512], BF16, tag=f"tl2_{i}",
                                       bufs=1, name=f"l2_{hh}_{lqc}_{i}")
                              for i in range(4)]
                        l3 = [att.tile([P, 512], BF16, tag=f"tl3_{i}",
                                       bufs=1, name=f"l3_{hh}_{lqc}_{i}")
                              for i in range(2)]
                        acc = att.tile([P, 512], BF16, tag="tacc", bufs=4,
                                       name=f"acc{hh}_{lqc}")
                        for lk in range(LT):
                            ps = aps.tile([P, 512], F32, tag="ps", bufs=3,
                                          name=f"ps{hh}_{lqc}_{lk}")
                            nc.tensor.matmul(ps[:],
                                             kT[0][:, lk * P:(lk + 1) * P],
                                             qh0[:, qsl],
                                             start=True, stop=False)
                            nc.tensor.matmul(ps[:],
                                             kT[1][:, lk * P:(lk + 1) * P],
                                             qh1[:, qsl],
                                             start=False, stop=True)
                            nc.scalar.activation(pt_t[lk][:], ps[:], AF.Exp,
                                                 scale=float(SCALING))
                            if lk % 2 == 1:
                                nc.vector.tensor_tensor(
                                    l1[lk // 2][:], pt_t[lk - 1][:],
                                    pt_t[lk][:], OP.add)
                            pop_units(2)
                        for i in range(4):
                            nc.vector.tensor_tensor(l2[i][:], l1[2 * i][:],
                                                    l1[2 * i + 1][:], OP.add)
                        for i in range(2):
                            nc.vector.tensor_tensor(l3[i][:], l2[2 * i][:],
                                                    l2[2 * i + 1][:], OP.add)
                        nc.vector.tensor_tensor(acc[:], l3[0][:], l3[1][:],
                                                OP.add)
                        backlog.extend(make_po_units(hh, lqc, pt_t, acc))
                        if lqc == 3:
                            backlog.extend(make_oproj_units(hh))
                # flush remaining deferred PE work
                pop_units(len(backlog))

    nc.compile()
    return nc


_NC = None


def _get_nc():
    global _NC
    if _NC is None:
        _NC = build_program()
    return _NC


def _bf16(a):
    return np.ascontiguousarray(a).astype(ml_dtypes.bfloat16)


def make_in_maps(hidden_states, cos, sin, Wq, Wk, Wv, Wo):
    hs = np.asarray(hidden_states, np.float32)
    xt = [_bf16(hs[b].T) for b in range(B)]
    cost = _bf16(np.asarray(cos, np.float32).T)
    sint = _bf16(np.asarray(sin, np.float32).T)
    wqt = np.ascontiguousarray(np.asarray(Wq, np.float32).T)
    wkvt = _bf16(np.concatenate(
        [np.asarray(Wk, np.float32).T, np.asarray(Wv, np.float32).T],
        axis=1))
    wot = _bf16(np.asarray(Wo, np.float32).T)
    in_maps = []
    for c in range(N_CORES):
        b, ql = c // 4, c % 4
        in_maps.append({
            "xt": xt[b],
            "cost": cost,
            "sint": sint,
            "wqt": _bf16(wqt[:, ql * 512:(ql + 1) * 512]),
            "wkvt": wkvt,
            "wot": wot,
        })
    return in_maps


def assemble(results):
    y = np.empty((B, L, E), np.float32)
    for c in range(N_CORES):
        b, ql = c // 4, c % 4
        y[b, ql * 512:(ql + 1) * 512, :] = results[c]["out"]
    return y


def kernel(hidden_states, attention_mask, cos, sin, Wq, Wk, Wv, Wo):
    # attention_mask is additive and all-zero per the problem spec; it is
    # accepted for signature compatibility but not shipped to the device.
    nc = _get_nc()
    in_maps = make_in_maps(hidden_states, cos, sin, Wq, Wk, Wv, Wo)
    res = run_bass_kernel_spmd(nc, in_maps, core_ids=list(range(N_CORES)))
    return assemble(res.results)


# revision 14
# speedup vs baseline: 1.1123x; 1.0281x over previous
"""ColPali MQA attention block on 8 Trainium2 NeuronCores.

The reference contains the ColPali reshape quirk: the attention output
[B, H, L, 1, D] is reshaped row-major straight to [B, L, H*D], which mixes
heads and positions.  Output row l' therefore depends ONLY on head
h = l'//256, gathering positions (l'%256)*8 + j for j in 0..7.

Sharding: core c -> batch b=c//4 and heads {h0, h0+1} with h0=2*(c%4).
Each core computes the K/V projection for its batch (replicated inside the
4-core batch group -- a cross-core AllGather was measured at ~100us
effective latency in this environment, worse than the 42us of replicated
PE work), Q projection + attention for its 2 heads, and o_proj for output
rows [512*ql, 512*ql+512).  Per-core outputs are disjoint slices -> no
cross-core communication.

All inputs are pre-converted to bf16 on the host and DMA'd straight into
their SBUF layouts (no on-device staging casts); Wk/Wv ship concatenated
so every DMA has >=1KB contiguous lines.  Phase 1 runs lc-chunk-major
(K -> V -> Q per 512-column chunk) so the PE never idles long enough for
the HAM clock gate to re-throttle.

The softmax row-sum is a pairwise bf16 add-tree on the vector engine plus
one all-ones matmul per block for the cross-partition reduction (instead
of 16 PE matmuls); 1/rowsum uses the fast approximate DVE reciprocal
(~18 bits, plenty against the 2e-2 gate).  Phase 2 is software-pipelined:
the scores+exp stream for attention block N is interleaved (2 matmuls per
score-pair) with a FIFO backlog of deferred PE work (PV accumulation of
block N-1, rowsum reduce, o_proj chains), so the PE never waits on the
scalar engine's exp drain.
"""

import numpy as np
import ml_dtypes

import concourse.mybir as mybir
import concourse.tile as tile
from concourse import bacc
from concourse.bass_utils import run_bass_kernel_spmd

F32 = mybir.dt.float32
BF16 = mybir.dt.bfloat16
AF = mybir.ActivationFunctionType
OP = mybir.AluOpType

B, L, H, D, E = 2, 2048, 8, 256, 2048
HD = H * D  # 2048
P = 128
EC = E // P  # 16 e-chunks
LT = L // P  # 16 l-tiles
SCALING = D ** -0.5  # 1/16
N_CORES = 8


def build_program():
    nc = bacc.Bacc("TRN2", target_bir_lowering=False, debug=False,
                   num_devices=N_CORES)

    xt = nc.dram_tensor("xt", [E, L], BF16, kind="ExternalInput").ap()
    cost = nc.dram_tensor("cost", [D, L], BF16, kind="ExternalInput").ap()
    sint = nc.dram_tensor("sint", [D, L], BF16, kind="ExternalInput").ap()
    wqt = nc.dram_tensor("wqt", [E, 2 * D], BF16, kind="ExternalInput").ap()
    # Wk and Wv concatenated on the output dim: [E, 2*D]
    wkvt = nc.dram_tensor("wkvt", [E, 2 * D], BF16,
                          kind="ExternalInput").ap()
    wot = nc.dram_tensor("wot", [HD, E], BF16, kind="ExternalInput").ap()
    out = nc.dram_tensor("out", [4 * P, E], F32, kind="ExternalOutput").ap()

    with tile.TileContext(nc) as tc:
        with tc.tile_pool(name="res", bufs=1) as res:
            kT = [res.tile([P, L], BF16, tag=f"kT{i}", name=f"kT{i}")
                  for i in range(2)]
            v_bf = [res.tile([P, D], BF16, tag=f"v{i}", name=f"v{i}")
                    for i in range(LT)]
            qT = [res.tile([P, L], BF16, tag=f"qT{i}", name=f"qT{i}")
                  for i in range(4)]
            ones128 = res.tile([P, P], BF16, tag="ones128", name="ones128")
            nc.vector.memset(ones128[:], 1.0)
            # pre-load the scalar engine's Exp table so the first real exp
            # in phase 2 doesn't pay the ACT_TABLE_LOAD (~1.3us)
            warm = res.tile([P, 8], F32, tag="warm", name="warm")
            nc.scalar.activation(warm[:], ones128[:, 0:8], AF.Exp)

            # ---------------- Phase 1: projections + RoPE ----------------
            with tc.tile_pool(name="proj", bufs=1) as proj, \
                 tc.tile_pool(name="proj_ps", space="PSUM", bufs=1) as pps:
                wkv = [proj.tile([P, 2 * D], BF16, tag=f"wkv{e}",
                                 name=f"wkv{e}") for e in range(EC)]
                wq = [proj.tile([P, 2 * D], BF16, tag=f"wq{e}",
                                name=f"wq{e}") for e in range(EC)]
                xf = [proj.tile([P, L], BF16, tag=f"xf{e}", name=f"xf{e}")
                      for e in range(EC)]
                # cos/sin tiles per (name, half, lc-pair): [128, 1024]
                csk = {}

                def load_csk(pair, nsplit=1):
                    sl = slice(pair * 1024, (pair + 1) * 1024)
                    for ci, (nm, srcd) in enumerate(
                            (("cos", cost), ("sin", sint))):
                        for half in range(2):
                            t = proj.tile([P, 1024], BF16, tag="csk",
                                          bufs=8, name=f"cs{nm}{half}{pair}")
                            eng = nc.sync if (ci + half) % 2 else nc.scalar
                            step = P // nsplit
                            for i in range(nsplit):
                                eng.dma_start(
                                    out=t[i * step:(i + 1) * step, :],
                                    in_=srcd[half * P + i * step:
                                             half * P + (i + 1) * step, sl])
                            csk[(nm, half, pair)] = t

                def dma_split(eng, dst, src, e, cols, n):
                    # partition-split one [128, w] transfer into n pieces so
                    # the first-needed tiles land via n parallel DMA queues
                    step = P // n
                    for i in range(n):
                        rs = slice(e * P + i * step, e * P + (i + 1) * step)
                        eng.dma_start(out=dst[i * step:(i + 1) * step,
                                              cols],
                                      in_=src[rs, cols])

                # first-needed DMAs first; 1-2KB contiguous lines
                # throughout.  Issue alternates between the sync and scalar
                # HWDGE queues (descriptor pushes from one engine
                # serialize), with the earliest chunks partition-split
                # across DMA queues.
                for e in range(EC):
                    n = 4 if e < 2 else (2 if e < 6 else 1)
                    ea, eb = (nc.sync, nc.scalar) if e % 2 else \
                        (nc.scalar, nc.sync)
                    dma_split(ea, wkv[e], wkvt, e, slice(0, 2 * D), n)
                    dma_split(eb, xf[e], xt, e, slice(0, 1024), n)
                    dma_split(ea, wq[e], wqt, e, slice(0, 2 * D),
                              2 if e < 6 else 1)
                    if e == 7:
                        load_csk(0, nsplit=2)
                for e in range(EC):
                    eng = nc.sync if e % 2 else nc.scalar
                    eng.dma_start(out=xf[e][:, 1024:2048],
                                  in_=xt[e * P:(e + 1) * P, 1024:2048])
                load_csk(1)

                def _rope(p0, p1, cs, out0, out1, tag):
                    ta = proj.tile([P, 512], F32, tag="ropetmp", bufs=4,
                                   name=f"ta{tag}")
                    tb = proj.tile([P, 512], F32, tag="ropetmp", bufs=4,
                                   name=f"tb{tag}")
                    nc.vector.tensor_tensor(ta[:], p0[:], cs[0], OP.mult)
                    nc.vector.tensor_tensor(tb[:], p1[:], cs[1], OP.mult)
                    nc.vector.tensor_tensor(out0, ta[:], tb[:], OP.subtract)
                    tc2 = proj.tile([P, 512], F32, tag="ropetmp", bufs=4,
                                    name=f"tc{tag}")
                    td = proj.tile([P, 512], F32, tag="ropetmp", bufs=4,
                                   name=f"td{tag}")
                    nc.vector.tensor_tensor(tc2[:], p1[:], cs[2], OP.mult)
                    nc.vector.tensor_tensor(td[:], p0[:], cs[3], OP.mult)
                    nc.vector.tensor_tensor(out1, tc2[:], td[:], OP.add)

                # K -> V -> Q per 512-column l-chunk: PE stays dense, each
                # chunk's inputs arrive while the previous chunk computes.
                for lc in range(4):
                    sl = slice(lc * 512, (lc + 1) * 512)
                    pair, ph = lc // 2, lc % 2
                    psl = slice(ph * 512, (ph + 1) * 512)
                    cs = [csk[("cos", 0, pair)][:, psl],
                          csk[("sin", 0, pair)][:, psl],
                          csk[("cos", 1, pair)][:, psl],
                          csk[("sin", 1, pair)][:, psl]]

                    pk0 = pps.tile([P, 512], F32, tag="pk", bufs=2,
                                   name=f"pk0_{lc}")
                    pk1 = pps.tile([P, 512], F32, tag="pk", bufs=2,
                                   name=f"pk1_{lc}")
                    for e in range(EC):
                        st, sp = (e == 0), (e == EC - 1)
                        xs = xf[e][:, sl]
                        nc.tensor.matmul(pk0[:], wkv[e][:, 0:P], xs,
                                         start=st, stop=sp)
                        nc.tensor.matmul(pk1[:], wkv[e][:, P:2 * P], xs,
                                         start=st, stop=sp)
                    _rope(pk0, pk1, cs, kT[0][:, sl], kT[1][:, sl],
                          f"k{lc}")

                    # head-A Q first: its rope retires off the vector queue
                    # while V/head-B still compute, so phase 2 (which needs
                    # kT + qT[0,1]) starts without waiting on vector.
                    pq = [pps.tile([P, 512], F32, tag=f"pq{j}", bufs=1,
                                   name=f"pq{lc}_{j}") for j in range(4)]
                    for e in range(EC):
                        st, sp = (e == 0), (e == EC - 1)
                        xs = xf[e][:, sl]
                        nc.tensor.matmul(pq[0][:], wq[e][:, 0:P], xs,
                                         start=st, stop=sp)
                        nc.tensor.matmul(pq[1][:], wq[e][:, P:2 * P], xs,
                                         start=st, stop=sp)
                    _rope(pq[0], pq[1], cs, qT[0][:, sl], qT[1][:, sl],
                          f"q0{lc}")

                    for lt in range(4 * lc, 4 * lc + 4):
                        pv = pps.tile([P, D], F32, tag="pv", bufs=2,
                                      name=f"pv{lt}")
                        for e in range(EC):
                            nc.tensor.matmul(
                                pv[:], xf[e][:, lt * P:(lt + 1) * P],
                                wkv[e][:, 2 * P:4 * P],
                                start=(e == 0), stop=(e == EC - 1))
                        nc.scalar.copy(v_bf[lt][:], pv[:])

                    for e in range(EC):
                        st, sp = (e == 0), (e == EC - 1)
                        xs = xf[e][:, sl]
                        nc.tensor.matmul(pq[2][:], wq[e][:, 2 * P:3 * P],
                                         xs, start=st, stop=sp)
                        nc.tensor.matmul(pq[3][:], wq[e][:, 3 * P:4 * P],
                                         xs, start=st, stop=sp)
                    _rope(pq[2], pq[3], cs, qT[2][:, sl], qT[3][:, sl],
                          f"q1{lc}")

            # ------------- Phase 2: attention + o_proj -------------
            with tc.tile_pool(name="att", bufs=1) as att, \
                 tc.tile_pool(name="att_ps", space="PSUM", bufs=1) as aps:
                G = [[att.tile([P, L], BF16, tag=f"G{hh}{dt}",
                               name=f"G{hh}{dt}") for dt in range(2)]
                     for hh in range(2)]
                wo = [att.tile([P, E], BF16, tag=f"wo{i}", name=f"wo{i}")
                      for i in range(EC)]
                for i in range(EC):
                    nc.sync.dma_start(out=wo[i][:],
                                      in_=wot[i * P:(i + 1) * P, :])

                backlog = []

                def pop_units(n):
                    for _ in range(n):
                        if backlog:
                            backlog.pop(0)()

                def make_po_units(hh, lqc, pt_t, acc):
                    """Deferred PV chains + rowsum reduce + normalize."""
                    cell = {}

                    def po_unit(dt, lk):
                        def u():
                            if lk == 0:
                                cell[dt] = aps.tile(
                                    [P, 512], F32, tag="po", bufs=2,
                                    name=f"po{hh}_{lqc}_{dt}")
                            nc.tensor.matmul(
                                cell[dt][:],
                                v_bf[lk][:, dt * P:(dt + 1) * P],
                                pt_t[lk][:],
                                start=(lk == 0), stop=(lk == LT - 1))
                        return u

                    def fin():
                        prb = aps.tile([P, 512], F32, tag="prb", bufs=1,
                                       name=f"prb{hh}_{lqc}")
                        nc.tensor.matmul(prb[:], ones128[:], acc[:],
                                         start=True, stop=True)
                        rb = att.tile([P, 512], F32, tag="rb", bufs=2,
                                      name=f"rb{hh}_{lqc}")
                        nc.vector.reciprocal_approx_fast(out=rb[:],
                                                         in_=prb[:])
                        rb_wu = rb.rearrange("p (u w) -> p w u", w=8)
                        for dt in range(2):
                            g_dst = G[hh][dt].rearrange(
                                "p (w r) -> p w r",
                                w=8)[:, :, 64 * lqc:64 * lqc + 64]
                            nc.vector.tensor_tensor(
                                g_dst,
                                cell[dt].rearrange("p (u w) -> p w u", w=8),
                                rb_wu, OP.mult)
                    units = [po_unit(dt, lk) for dt in range(2)
                             for lk in range(LT)]
                    units.append(fin)
                    return units

                def make_oproj_units(a_idx):
                    units = []
                    for rh in range(2):
                        rt = a_idx * 2 + rh
                        for eg in range(4):
                            esl = slice(eg * 512, (eg + 1) * 512)
                            cell = {}

                            def mm_unit(m, rt=rt, rh=rh, eg=eg, esl=esl,
                                        cell=cell):
                                def u():
                                    if m == 0:
                                        cell["py"] = aps.tile(
                                            [P, 512], F32, tag="py", bufs=2,
                                            name=f"py{rt}_{eg}")
                                    lhsT = G[a_idx][m % 2][
                                        :, (m // 2) * 256 + rh * P:
                                           (m // 2) * 256 + rh * P + P]
                                    nc.tensor.matmul(cell["py"][:], lhsT,
                                                     wo[m][:, esl],
                                                     start=(m == 0),
                                                     stop=(m == EC - 1))
                                    if m == EC - 1:
                                        ysb = att.tile(
                                            [P, 512], F32, tag="ysb",
                                            bufs=3, name=f"ysb{rt}_{eg}")
                                        eng = (nc.scalar.copy if eg % 2
                                               else nc.vector.tensor_copy)
                                        eng(ysb[:], cell["py"][:])
                                        # 2-way split -> 2 parallel queues
                                        for i in range(2):
                                            rsl = slice(
                                                rt * P + i * 64,
                                                rt * P + (i + 1) * 64)
                                            nc.sync.dma_start(
                                                out=out[rsl, esl],
                                                in_=ysb[i * 64:(i + 1) * 64,
                                                        :])
                                return u
                            units.extend(mm_unit(m) for m in range(EC))
                    return units

                for hh in range(2):
                    qh0, qh1 = qT[2 * hh], qT[2 * hh + 1]
                    for lqc in range(4):
                        qsl = slice(lqc * 512, (lqc + 1) * 512)
                        pt_t = [att.tile([P, 512], BF16, tag=f"pt{i}",
                                         bufs=4, name=f"pt{hh}_{lqc}_{i}")
                                for i in range(LT)]
                        l1 = [att.tile([P, 512], BF16, tag=f"tl1_{i}",
                                       bufs=1, name=f"l1_{hh}_{lqc}_{i}")
                              for i in range(8)]
                        l2 = [att.tile([P, 512], BF16, tag=f"tl2_{i}",
                                       bufs=1, name=f"l2_{hh}_{lqc}_{i}")
                              for i in range(4)]
                        l3 = [att.tile([P, 512], BF16, tag=f"tl3_{i}",
                                       bufs=1, name=f"l3_{hh}_{lqc}_{i}")
                              for i in range(2)]
                        acc = att.tile([P, 512], BF16, tag="tacc", bufs=4,
                                       name=f"acc{hh}_{lqc}")
                        for lk in range(LT):
                            ps = aps.tile([P, 512], F32, tag="ps", bufs=3,
                                          name=f"ps{hh}_{lqc}_{lk}")
                            nc.tensor.matmul(ps[:],
                                             kT[0][:, lk * P:(lk + 1) * P],
                                             qh0[:, qsl],
                                             start=True, stop=False)
                            nc.tensor.matmul(ps[:],
                                             kT[1][:, lk * P:(lk + 1) * P],
                                             qh1[:, qsl],
                                             start=False, stop=True)
                            nc.scalar.activation(pt_t[lk][:], ps[:], AF.Exp,
                                                 scale=float(SCALING))
                            if lk % 2 == 1:
                                nc.vector.tensor_tensor(
                                    l1[lk // 2][:], pt_t[lk - 1][:],
                                    pt_t[lk][:], OP.add)
                            pop_units(2)
                        for i in range(4):
                            nc.vector.tensor_tensor(l2[i][:], l1[2 * i][:],
                                                    l1[2 * i + 1][:], OP.add)
                        for i in range(2):
                            nc.vector.tensor_tensor(l3[i][:], l2[2 * i][:],
                                                    l2[2 * i + 1][:], OP.add)
                        nc.vector.tensor_tensor(acc[:], l3[0][:], l3[1][:],
                                                OP.add)
                        backlog.extend(make_po_units(hh, lqc, pt_t, acc))
                        if lqc == 3:
                            backlog.extend(make_oproj_units(hh))
                # flush remaining deferred PE work
                pop_units(len(backlog))

    nc.compile()
    return nc


_NC = None


def _get_nc():
    global _NC
    if _NC is None:
        _NC = build_program()
    return _NC


def _bf16(a):
    return np.ascontiguousarray(a).astype(ml_dtypes.bfloat16)


def make_in_maps(hidden_states, cos, sin, Wq, Wk, Wv, Wo):
    hs = np.asarray(hidden_states, np.float32)
    xt = [_bf16(hs[b].T) for b in range(B)]
    cost = _bf16(np.asarray(cos, np.float32).T)
    sint = _bf16(np.asarray(sin, np.float32).T)
    wqt = np.ascontiguousarray(np.asarray(Wq, np.float32).T)
    wkvt = _bf16(np.concatenate(
        [np.asarray(Wk, np.float32).T, np.asarray(Wv, np.float32).T],
        axis=1))
    wot = _bf16(np.asarray(Wo, np.float32).T)
    in_maps = []
    for c in range(N_CORES):
        b, ql = c // 4, c % 4
        in_maps.append({
            "xt": xt[b],
            "cost": cost,
            "sint": sint,
            "wqt": _bf16(wqt[:, ql * 512:(ql + 1) * 512]),
            "wkvt": wkvt,
            "wot": wot,
        })
    return in_maps


def assemble(results):
    y = np.empty((B, L, E), np.float32)
    for c in range(N_CORES):
        b, ql = c // 4, c % 4
        y[b, ql * 512:(ql + 1) * 512, :] = results[c]["out"]
    return y


def kernel(hidden_states, attention_mask, cos, sin, Wq, Wk, Wv, Wo):
    # attention_mask is additive and all-zero per the problem spec; it is
    # accepted for signature compatibility but not shipped to the device.
    nc = _get_nc()
    in_maps = make_in_maps(hidden_states, cos, sin, Wq, Wk, Wv, Wo)
    res = run_bass_kernel_spmd(nc, in_maps, core_ids=list(range(N_CORES)))
    return assemble(res.results)
